# revision 4
# baseline (speedup 1.0000x reference)
"""NeRF (coarse+fine) forward pass on 8 TRN2 NeuronCores.

Strategy: pure data parallel over rays (128 rays/core). The two radiance-field
MLP evaluations (the arithmetic bulk: ~34 GFLOP) run on device as fp32r
matmuls with points on the moving free axis (blocks of 512 points) and
features on partitions. Everything cheap and precision-critical — stratified /
importance sampling, positional encodings, volumetric compositing — runs on
host with jax (CPU) using formulas identical to the reference, so the only
divergence from the fp32 reference is the fp32r matmul rounding (~2e-4).

Device kernel outputs raw sigma/rgb pre-activations; host applies
relu/sigmoid/compositing, then importance-samples t_f from the device-computed
coarse weights and launches the fine pass.
"""

import numpy as np
import jax
import jax.numpy as jnp

import concourse.bass as bass
import concourse.tile as tile
from concourse import bacc, mybir
from concourse.bass_utils import run_bass_kernel_spmd

dt = mybir.dt
AF = mybir.ActivationFunctionType
ALU = mybir.AluOpType

# ---- problem constants (hardcoded per contract) ----
T_N, T_F = 0.0, 2.5
N_C, N_F = 64, 128
L_X, L_D = 10, 4
WIDTH = 256
B = 1024
N_CORES = 8
BR = B // N_CORES          # rays per core = 128
NB = 512                   # points per device block
DIN_X, DIN_D = 6 * L_X, 6 * L_D  # 60, 24


# ======================= host-side math (matches reference) =================

_CPU = None


def _cpu(x):
    """Commit an array to the CPU backend so jnp ops on it stay off axon."""
    global _CPU
    if _CPU is None:
        _CPU = jax.devices("cpu")[0]
    return jax.device_put(x, _CPU)


def _gamma(p, L):
    freqs = (2.0 ** jnp.arange(L, dtype=jnp.float32)) * jnp.pi
    ang = p[..., None, :] * freqs[:, None]
    enc = jnp.concatenate([jnp.sin(ang), jnp.cos(ang)], axis=-1)
    return enc.reshape(p.shape[:-1] + (6 * L,))


def _sample_coarse(key, partitions):
    lo, hi = partitions[:, :-1], partitions[:, 1:]
    return lo + (hi - lo) * jax.random.uniform(key, lo.shape, dtype=partitions.dtype)


def _sample_fine(key, partitions, w, t_c, N_f):
    w = jnp.maximum(w, 1e-16)
    pdf = w / jnp.sum(w, axis=1, keepdims=True)
    cdf = jnp.concatenate([jnp.zeros_like(pdf[:, :1]), jnp.cumsum(pdf, axis=1)], axis=1)
    u = jax.random.uniform(key, (w.shape[0], N_f), dtype=w.dtype)
    idx = jax.vmap(lambda c, uu: jnp.searchsorted(c, uu, side='right'))(cdf, u)
    idx = jnp.clip(idx, 1, w.shape[1])
    cdf_lo = jnp.take_along_axis(cdf, idx - 1, axis=1)
    cdf_hi = jnp.take_along_axis(cdf, idx, axis=1)
    t_lo = jnp.take_along_axis(partitions, idx - 1, axis=1)
    t_hi = jnp.take_along_axis(partitions, idx, axis=1)
    denom = jnp.where(cdf_hi - cdf_lo < 1e-10, 1.0, cdf_hi - cdf_lo)
    t_s = t_lo + (u - cdf_lo) / denom * (t_hi - t_lo)
    return jnp.sort(jnp.concatenate([t_c, t_s], axis=1), axis=1)


def _composite(rgb, sigma, t):
    # rgb (B,N,3), sigma (B,N), t (B,N) -> C (B,3), w (B,N)
    Bn = t.shape[0]
    delta = jnp.concatenate([t[:, 1:] - t[:, :-1],
                             jnp.full((Bn, 1), 1e8, t.dtype)], axis=1)
    mass = sigma * delta
    alpha = 1.0 - jnp.exp(-mass)
    T = jnp.exp(-jnp.cumsum(
        jnp.concatenate([jnp.zeros((Bn, 1), t.dtype), mass[:, :-1]], axis=1), axis=1))
    w = T * alpha
    C = jnp.sum(w[..., None] * rgb, axis=1) + (1.0 - jnp.sum(w, axis=1, keepdims=True))
    return C, w


# ======================= device program =====================================

# hidden layer weight chunking plan. Weight arrays are (K_in, M_out); the PE
# stationary operand lhsT is exactly a [K<=128, M<=128] slice of that.

def _build_mlp_program(P):
    """MLP evaluator for P points (P % 512 == 0), one radiance field.

    DRAM in : gx [60,P], gdrep [24,512], weight chunks, bias_all [128,19]
    DRAM out: sigz [1,P] (pre-relu sigma), rgbz [3,P] (pre-sigmoid rgb)
    """
    nblk = P // NB
    nc = bacc.Bacc("TRN2", target_bir_lowering=False, debug=False,
                   num_devices=N_CORES)
    f32, f32r = dt.float32, dt.float32r

    gx_d = nc.dram_tensor("gx", [DIN_X, P], f32, kind="ExternalInput")
    gdrep_d = nc.dram_tensor("gdrep", [DIN_D, NB], f32, kind="ExternalInput")
    bias_d = nc.dram_tensor("bias_all", [128, 19], f32, kind="ExternalInput")
    sigz_d = nc.dram_tensor("sigz", [1, P], f32, kind="ExternalOutput")
    rgbz_d = nc.dram_tensor("rgbz", [3, P], f32, kind="ExternalOutput")

    # weight chunk declarations: name -> (K, M)
    wspec = {}
    for m in range(2):
        wspec[f"W0_m{m}"] = (DIN_X, 128)
    for li in list(range(1, 8)) + ["feat"]:
        for k in range(2):
            for m in range(2):
                wspec[f"W{li}_k{k}_m{m}"] = (128, 128)
    for m in range(2):
        wspec[f"W5_gx_m{m}"] = (DIN_X, 128)     # skip-concat gx part of W5
    for k in range(2):
        wspec[f"Wsig_k{k}"] = (128, 1)
    for k in range(2):
        wspec[f"Wrgb1_k{k}"] = (128, 128)
    wspec["Wrgb1_gd"] = (DIN_D, 128)
    wspec["Wrgb2"] = (128, 3)

    wd = {n: nc.dram_tensor(n, list(sh), f32, kind="ExternalInput")
          for n, sh in wspec.items()}

    with tile.TileContext(nc) as tc:
        with tc.tile_pool(name="wpool", bufs=1) as wp, \
             tc.tile_pool(name="dpool", bufs=3) as dp, \
             tc.tile_pool(name="hpool", bufs=2) as hp, \
             tc.tile_pool(name="opool", bufs=3) as op, \
             tc.tile_pool(name="ppool", bufs=6, space="PSUM") as pp, \
             tc.tile_pool(name="spool", bufs=1, space="PSUM") as sp:

            # ---- persistent tiles: weights (fp32r), biases, gdrep
            wt = {}
            for n, (K, M) in wspec.items():
                wt[n] = wp.tile([K, M], f32r, tag=n, name=n)
                nc.sync.dma_start(wt[n][:], wd[n][:].bitcast(f32r))
            bias_t = wp.tile([128, 19], f32, tag="bias", name="bias")
            nc.sync.dma_start(bias_t[:], bias_d[:])
            gdrep_t = wp.tile([DIN_D, NB], f32r, tag="gdrep", name="gdrep")
            nc.sync.dma_start(gdrep_t[:], gdrep_d[:].bitcast(f32r))

            # bias column index per (layer, mchunk): layers 0..7 -> 0..15,
            # feat -> 16,17, rgb1 -> 18 (all zeros in practice).
            def bcol(j):
                return bias_t[:, j:j + 1]

            for b in range(nblk):
                lo = b * NB

                gxt = dp.tile([DIN_X, NB], f32r, tag="gx", name="gxt")
                nc.sync.dma_start(gxt[:], gx_d[:, lo:lo + NB].bitcast(f32r))

                # dense layer helper: ins = list of (tile, K) K-chunks
                def dense(ins, wnames, bias_cols, relu=True, engine="act",
                          tagp="h"):
                    outs = []
                    for m in range(2):
                        p = pp.tile([128, NB], f32, tag="mm", name="pmm")
                        nk = len(ins)
                        for j, (kt, K) in enumerate(ins):
                            nc.tensor.matmul(p[:], wt[wnames[m][j]][:],
                                             kt[:K, :],
                                             start=(j == 0), stop=(j == nk - 1))
                        h = hp.tile([128, NB], f32r, tag=f"{tagp}{m}",
                                    name="ht")
                        if engine == "act":
                            nc.scalar.activation(
                                h[:], p[:], AF.Relu if relu else AF.Identity,
                                bias=bcol(bias_cols[m]), scale=1.0)
                        else:
                            if relu:
                                nc.vector.tensor_scalar(
                                    h[:], p[:], bcol(bias_cols[m]), 0.0,
                                    ALU.add, ALU.max)
                            else:
                                nc.vector.tensor_scalar(
                                    h[:], p[:], bcol(bias_cols[m]), None,
                                    ALU.add)
                        outs.append(h)
                    return outs

                # L0: gx -> h
                h = dense([(gxt, DIN_X)],
                          [[f"W0_m{m}"] for m in range(2)],
                          [0, 1], tagp="h")
                # L1..L4
                for li in range(1, 5):
                    h = dense([(h[0], 128), (h[1], 128)],
                              [[f"W{li}_k0_m{m}", f"W{li}_k1_m{m}"]
                               for m in range(2)],
                              [2 * li, 2 * li + 1], tagp="h")
                # L5: skip concat [h, gx]
                h = dense([(h[0], 128), (h[1], 128), (gxt, DIN_X)],
                          [[f"W5_k0_m{m}", f"W5_k1_m{m}", f"W5_gx_m{m}"]
                           for m in range(2)],
                          [10, 11], tagp="h")
                # L6, L7
                for li in range(6, 8):
                    h = dense([(h[0], 128), (h[1], 128)],
                              [[f"W{li}_k0_m{m}", f"W{li}_k1_m{m}"]
                               for m in range(2)],
                              [2 * li, 2 * li + 1], tagp="h")

                # sigma head: [1, NB] psum (no bias, no relu on device)
                psig = sp.tile([1, NB], f32, tag="psig", name="psig")
                nc.tensor.matmul(psig[:], wt["Wsig_k0"][:], h[0][:],
                                 start=True, stop=False)
                nc.tensor.matmul(psig[:], wt["Wsig_k1"][:], h[1][:],
                                 start=False, stop=True)
                sig_s = op.tile([1, NB], f32, tag="sig", name="sig_s")
                nc.vector.tensor_copy(sig_s[:], psig[:])
                nc.sync.dma_start(sigz_d[:, lo:lo + NB], sig_s[:])

                # feat (no relu), evacuated on DVE to keep ACT free
                feat = dense([(h[0], 128), (h[1], 128)],
                             [["Wfeat_k0_m0", "Wfeat_k1_m0"],
                              ["Wfeat_k0_m1", "Wfeat_k1_m1"]],
                             [16, 17], relu=False, engine="dve", tagp="f")

                # rgb1: [feat, gdrep] -> h2 (relu)
                p1 = pp.tile([128, NB], f32, tag="mm", name="p1")
                nc.tensor.matmul(p1[:], wt["Wrgb1_k0"][:], feat[0][:],
                                 start=True, stop=False)
                nc.tensor.matmul(p1[:], wt["Wrgb1_k1"][:], feat[1][:],
                                 start=False, stop=False)
                nc.tensor.matmul(p1[:], wt["Wrgb1_gd"][:], gdrep_t[:],
                                 start=False, stop=True)
                h2 = hp.tile([128, NB], f32r, tag="h2", name="h2")
                nc.scalar.activation(h2[:], p1[:], AF.Relu, bias=bcol(18),
                                     scale=1.0)

                # rgb2 -> [3, NB] raw
                prgb = sp.tile([3, NB], f32, tag="prgb", name="prgb")
                nc.tensor.matmul(prgb[:], wt["Wrgb2"][:], h2[:],
                                 start=True, stop=True)
                rgb_s = op.tile([3, NB], f32, tag="rgb", name="rgb_s")
                nc.vector.tensor_copy(rgb_s[:], prgb[:])
                nc.sync.dma_start(rgbz_d[:, lo:lo + NB], rgb_s[:])

    nc.compile()
    return nc


_PROGRAM_CACHE = {}


def _get_program(P):
    if P not in _PROGRAM_CACHE:
        _PROGRAM_CACHE[P] = _build_mlp_program(P)
    return _PROGRAM_CACHE[P]


# ======================= host <-> device glue ===============================

def _chunk_params(params):
    """Split reference param dict into the DRAM chunk arrays (shared by all
    cores) + bias_all [128,19]."""
    g = {k: np.asarray(v, np.float32) for k, v in params.items()}
    out = {}
    W0 = g["W0"]                       # (60, 256)
    for m in range(2):
        out[f"W0_m{m}"] = np.ascontiguousarray(W0[:, m * 128:(m + 1) * 128])
    for li in range(1, 8):
        W = g[f"W{li}"]
        if li == 5:                    # (316, 256): rows 0:256 h, 256:316 gx
            for k in range(2):
                for m in range(2):
                    out[f"W5_k{k}_m{m}"] = np.ascontiguousarray(
                        W[k * 128:(k + 1) * 128, m * 128:(m + 1) * 128])
            for m in range(2):
                out[f"W5_gx_m{m}"] = np.ascontiguousarray(
                    W[256:, m * 128:(m + 1) * 128])
        else:
            for k in range(2):
                for m in range(2):
                    out[f"W{li}_k{k}_m{m}"] = np.ascontiguousarray(
                        W[k * 128:(k + 1) * 128, m * 128:(m + 1) * 128])
    Wf = g["W_feat"]
    for k in range(2):
        for m in range(2):
            out[f"Wfeat_k{k}_m{m}"] = np.ascontiguousarray(
                Wf[k * 128:(k + 1) * 128, m * 128:(m + 1) * 128])
    Ws = g["W_sigma"]                  # (256,1)
    for k in range(2):
        out[f"Wsig_k{k}"] = np.ascontiguousarray(Ws[k * 128:(k + 1) * 128])
    W1r = g["W_rgb1"]                  # (280,128): rows 0:256 feat, 256:280 gd
    for k in range(2):
        out[f"Wrgb1_k{k}"] = np.ascontiguousarray(W1r[k * 128:(k + 1) * 128])
    out["Wrgb1_gd"] = np.ascontiguousarray(W1r[256:])
    out["Wrgb2"] = np.ascontiguousarray(g["W_rgb2"])  # (128,3)

    bias = np.zeros((128, 19), np.float32)
    for li in range(8):
        bb = g[f"b{li}"]
        bias[:, 2 * li] = bb[:128]
        bias[:, 2 * li + 1] = bb[128:]
    bias[:, 16] = g["b_feat"][:128]
    bias[:, 17] = g["b_feat"][128:]
    bias[:, 18] = g["b_rgb1"]
    return out, bias, g


def _run_mlp(params, gx_cores, gd_rays_cores):
    """Run the device MLP for all cores.

    gx_cores: list of [60, P] fp32 per core (sample-major points)
    gd_rays_cores: list of [BR, 24] per core
    returns sigz [N_CORES, P], rgbz [N_CORES, 3, P]
    """
    P = gx_cores[0].shape[1]
    nc = _get_program(P)
    wchunks, bias, _ = _chunk_params(params)
    in_maps = []
    for c in range(N_CORES):
        gdrep = np.ascontiguousarray(
            np.tile(gd_rays_cores[c].T, (1, NB // BR)).astype(np.float32))
        m = {"gx": np.ascontiguousarray(gx_cores[c]),
             "gdrep": gdrep, "bias_all": bias}
        m.update(wchunks)
        in_maps.append(m)
    res = run_bass_kernel_spmd(nc, in_maps, list(range(N_CORES)))
    sigz = np.stack([r["sigz"][0] for r in res.results])       # (8, P)
    rgbz = np.stack([r["rgbz"] for r in res.results])          # (8, 3, P)
    return sigz, rgbz


def _encode_points(o, d, t):
    """Host positional encodings, sharded+flattened sample-major per core.

    returns gx_cores (list of [60, P]), with P = Ns * BR, point p = s*BR + r.
    """
    Ns = t.shape[1]
    x = o[:, None, :] + t[..., None] * d[:, None, :]           # (B, Ns, 3)
    gx = _gamma(_cpu(x), L_X)                                  # (B, Ns, 60)
    gx = np.asarray(gx, np.float32)
    gx_cores = []
    for c in range(N_CORES):
        sl = gx[c * BR:(c + 1) * BR]                           # (BR, Ns, 60)
        # sample-major: [Ns, BR, 60] -> (P, 60) -> [60, P]
        arr = sl.transpose(1, 0, 2).reshape(Ns * BR, 60).T
        gx_cores.append(np.ascontiguousarray(arr))
    return gx_cores


def _unshard(sigz, rgbz, Ns):
    """Back to (B, Ns) sigma-z and (B, Ns, 3) rgb-z."""
    sig = np.concatenate(
        [sigz[c].reshape(Ns, BR).T for c in range(N_CORES)], axis=0)
    rgb = np.concatenate(
        [rgbz[c].reshape(3, Ns, BR).transpose(2, 1, 0) for c in range(N_CORES)],
        axis=0)
    return sig, rgb


def kernel(o, d, params_c, params_f):
    o = np.asarray(o, np.float32)
    d = np.asarray(d, np.float32)

    k_c, k_f = jax.random.split(_cpu(jax.random.key(42)))
    partitions = _cpu(jnp.broadcast_to(
        jnp.linspace(T_N, T_F, N_C + 1, dtype=np.float32), (B, N_C + 1)))
    t_c = np.asarray(_sample_coarse(k_c, partitions))

    gd = np.asarray(_gamma(_cpu(d), L_D), np.float32)          # (B, 24)
    gd_cores = [gd[c * BR:(c + 1) * BR] for c in range(N_CORES)]

    # ---- coarse pass
    gx_c = _encode_points(o, d, t_c)
    sigz, rgbz = _run_mlp(params_c, gx_c, gd_cores)
    sig_z, rgb_z = _unshard(sigz, rgbz, N_C)
    bsig = float(np.asarray(params_c["b_sigma"], np.float32)[0])
    brgb = np.asarray(params_c["b_rgb2"], np.float32)
    sigma_c = jax.nn.relu(_cpu(sig_z) + bsig)
    rgb_c = jax.nn.sigmoid(_cpu(rgb_z) + _cpu(brgb))
    C_c, w_c = _composite(rgb_c, sigma_c, _cpu(t_c))

    # ---- fine sampling from device-computed coarse weights
    t_f = np.asarray(_sample_fine(k_f, partitions, w_c, _cpu(t_c), N_F))

    # ---- fine pass
    gx_f = _encode_points(o, d, t_f)
    sigz, rgbz = _run_mlp(params_f, gx_f, gd_cores)
    sig_z, rgb_z = _unshard(sigz, rgbz, N_C + N_F)
    bsig = float(np.asarray(params_f["b_sigma"], np.float32)[0])
    brgb = np.asarray(params_f["b_rgb2"], np.float32)
    sigma_f = jax.nn.relu(_cpu(sig_z) + bsig)
    rgb_f = jax.nn.sigmoid(_cpu(rgb_z) + _cpu(brgb))
    C_f, _ = _composite(rgb_f, sigma_f, _cpu(t_f))

    return np.asarray(C_c), np.asarray(C_f)
